# revision 2
# baseline (speedup 1.0000x reference)
"""Trainium2 Bass kernel for nn_Attention_661424964229.

Reference computation (x: [8, 4096] f32):
    y = ((x @ x^T) / 16) @ x   per batch row, which algebraically equals
    out[b, :] = x[b, :] * sum(x[b, :]**2) / 16

Sharding: pure data parallel — row b of the batch goes to core b (B=8 rows,
8 NeuronCores). Each core:
  1. DMA its row, viewed as [128, 32], HBM -> SBUF
  2. DVE tensor_tensor_reduce: sq = (x*x)/16, per-partition sums ss [128,1]
  3. PE matmul with an all-ones [128,128] stationary: PSUM [128,1] holds
     S/16 broadcast to every partition (1.0 * ss[k] is exact in fp32)
  4. DVE tensor_scalar_mul: res = x * (S/16)
  5. DMA res SBUF -> HBM
"""

import numpy as np

B, L = 8, 4096
P, F = 128, 32  # per-core row viewed as [128 partitions, 32 elems]

_cached = {}


def _build_program():
    import concourse.bass as bass
    from concourse import mybir

    nc = bass.Bass("TRN2", target_bir_lowering=False, debug=False)

    x_dram = nc.dram_tensor("x", [P, F], mybir.dt.float32, kind="ExternalInput")
    out_dram = nc.dram_tensor("out", [P, F], mybir.dt.float32, kind="ExternalOutput")

    with (
        nc.semaphore("dma_sem") as dma_sem,
        nc.semaphore("v_sem") as v_sem,
        nc.sbuf_tensor("xt", [P, F], mybir.dt.float32) as xt,
        nc.sbuf_tensor("sq", [P, F], mybir.dt.float32) as sq,
        nc.sbuf_tensor("ss", [P, 1], mybir.dt.float32) as ss,
        nc.sbuf_tensor("ones", [P, P], mybir.dt.float32) as ones,
        nc.sbuf_tensor("res", [P, F], mybir.dt.float32) as res,
        nc.psum_tensor("sb", [P, 1], mybir.dt.float32) as sb,
    ):
        with nc.Block() as block:

            @block.sync
            def _(sync):
                sync.dma_start(out=xt[:], in_=x_dram[:]).then_inc(dma_sem, 16)

            @block.vector
            def _(vector):
                # ones does not depend on the input DMA; overlap with it
                vector.memset(ones[:], 1.0)
                vector.wait_ge(dma_sem, 16)
                # sq = (x/16)*x ; ss[p] = sum_f sq[p, f]
                vector.scalar_tensor_tensor(
                    out=sq[:],
                    in0=xt[:],
                    scalar=0.0625,
                    in1=xt[:],
                    op0=bass.mybir.AluOpType.mult,
                    op1=bass.mybir.AluOpType.mult,
                    accum_out=ss[:],
                ).then_inc(v_sem, 1)
                # wait for PE broadcast-sum, then scale the row
                vector.wait_ge(v_sem, 2)
                vector.tensor_scalar_mul(res[:], xt[:], sb[:]).then_inc(v_sem, 1)

            @block.tensor
            def _(tensor):
                tensor.wait_ge(v_sem, 1)
                # sb[p, 0] = sum_k 1.0 * ss[k, 0]  (same value in every partition)
                tensor.matmul(sb[:], ones[:], ss[:], start=True, stop=True).then_inc(
                    v_sem, 1
                )

            @block.scalar
            def _(scalar):
                scalar.wait_ge(v_sem, 3)
                scalar.dma_start(out=out_dram[:], in_=res[:]).then_inc(dma_sem, 16)
                scalar.wait_ge(dma_sem, 32)

    return nc


def _get_nc():
    if "nc" not in _cached:
        _cached["nc"] = _build_program()
    return _cached["nc"]


def _run(x, trace=False, trace_kwargs=None):
    from concourse.bass_utils import run_bass_kernel_spmd

    nc = _get_nc()
    in_maps = [{"x": np.ascontiguousarray(x[b].reshape(P, F))} for b in range(B)]
    r = run_bass_kernel_spmd(
        nc,
        in_maps,
        core_ids=list(range(B)),
        trace=trace,
        **(trace_kwargs or {}),
    )
    out = np.empty((B, L), dtype=np.float32)
    for b in range(B):
        out[b] = r.results[b]["out"].reshape(L)
    return out, r


def kernel(x: np.ndarray) -> np.ndarray:
    out, _ = _run(np.asarray(x, dtype=np.float32))
    return out
